# revision 38
# baseline (speedup 1.0000x reference)
"""Expert-parallel MoE (Mixtral-style top-2 of 8 experts, SwiGLU) on 8 TRN2 cores.

Strategy: expert PAIRING with a half-intermediate split. Experts are sorted by
token load and paired big-with-small; the two cores of pair p each process ALL
tokens of both experts, but only half of the intermediate dim I (so weight
bytes per core stay at 1/8 of the total). The host sums the two half-I
down-proj partials per expert. This cuts the per-core matmul column count from
max_e(load) to maxA+maxB over the pair slots; slot capacities are uniform
across cores (SPMD single program): CA = max big-expert load, CB = max
small-expert load.

Routing (softmax/top-k/renorm, 1024x8) runs on host during input sharding;
the renormalized routing weight is folded in at host combine time.

Device kernel per core (all matmuls bf16, fp32 PSUM accumulation):
  GU:   for j, for slot in (A,B): g/u[j] = W13T[slot,j] . xt  (16 ko steps)
        act = silu(g)*u  (bf16); 2 PSUM accumulators per slot fold the first
        2 down-proj output chunks in as act[j-1] becomes available
  DOWN: remaining 14 output chunks per slot, 6-wide PSUM passes
        (reusing the g/u + accumulator banks); output written back as bf16.

DMA discipline: the sync engine issues each dma_start in ~650 ns, so the
stream is organized to need few instructions early (ko-block split of the
first gate tile interleaved with 4 xt block loads), and w2a/w2b transfers are
packed (both slots / j-pairs) to halve instruction count. Emission order ==
consumption order; w2b_A is paced into the stream over the back half of GU.
"""

import os

import ml_dtypes
import numpy as np

import concourse.bass as bass
from concourse import bacc
import concourse.mybir as mybir
import concourse.tile as tile
from concourse.bass_utils import run_bass_kernel_spmd

P = 128
H = 2048          # hidden dim
I = 4096          # intermediate dim
IH = I // 2       # per-core intermediate half
E = 8             # experts
N_CORES = 8
BF16 = mybir.dt.bfloat16
F32 = mybir.dt.float32

KO = H // P       # 16 contraction steps over hidden dim
NJ = IH // P      # 16 j-tiles per slot (half intermediate)
NH = H // P       # 16 output row chunks
NC1 = 2           # down-proj chunks folded into the GU phase per slot
XB = 8            # first-tile/xtA ko-blocks (smaller first chunks start the
                  # arrival-paced j0 region earlier; bytes unchanged)
WB = H - NC1 * P  # w2b column width

# set by kernel() for test harness introspection
last_results = None


def _build_nc(CA: int, CB: int) -> bass.Bass:
    act_fn = mybir.ActivationFunctionType
    CT = CA + CB

    nc = bacc.Bacc()
    xta_d = nc.declare_dram_parameter("xta", [P, KO, CA], BF16, isOutput=False)
    xtb_d = nc.declare_dram_parameter("xtb", [P, KO, CB], BF16, isOutput=False)
    # per slot: j-tiles 0..NJ-1 = gate, NJ..2NJ-1 = up
    w13_d = nc.declare_dram_parameter("w13", [2, 2 * NJ, P, KO, P], BF16, isOutput=False)
    # w2a packed across slots per j; w2b packed across j-pairs per slot
    w2a_d = nc.declare_dram_parameter("w2a", [NJ, P, 2, NC1 * P], BF16, isOutput=False)
    w2b_d = nc.declare_dram_parameter("w2b", [2, NJ // 2, P, 2, WB], BF16, isOutput=False)
    y_d = nc.declare_dram_parameter("y", [NH, P, CT], BF16, isOutput=True)

    SLOTS = [(0, 0, CA), (1, CA, CB)]  # (slot, col offset, width)

    with tile.TileContext(nc) as tc:
        with (
            tc.tile_pool(name="xp", bufs=1) as xp,
            tc.tile_pool(name="w13p", bufs=6) as w13p,
            tc.tile_pool(name="w2ap", bufs=4) as w2ap,
            tc.tile_pool(name="w2bp", bufs=1) as w2bp,
            tc.tile_pool(name="actp", bufs=1) as actp,
            tc.tile_pool(name="silup", bufs=2) as silup,
            tc.tile_pool(name="outp", bufs=4) as outp,
            tc.tile_pool(name="psgu", bufs=2, space="PSUM") as psgu,
            tc.tile_pool(name="psacc", bufs=1, space="PSUM") as psacc,
        ):
            w13_tiles = {}  # (slot, kind, j) -> sbuf tile; kind 0=gate 1=up

            w13_first = []  # j0 A-gate as 4 SEPARATE tiles: each ko-block gets
            # its own DMA semaphore, so matmul ko waits only its own 128 KB
            # chunk (a sliced single tile makes the first matmul wait for the
            # whole 512 KB tile)

            def dma_w13(slot, kind, j, blocked=False):
                src = w13_d[slot, kind * NJ + j]
                if blocked:
                    # ko-blocks interleaved with A-slot xt loads (scalar queue
                    # issues xt in parallel with the weight stream on sync)
                    kb = KO // XB
                    for b in range(XB):
                        wb_sb = w13p.tile([P, kb, P], BF16, tag=f"w13f_{b}", name=f"w13f_{b}")
                        nc.sync.dma_start(wb_sb[:], src[:, b * kb:(b + 1) * kb, :])
                        w13_first.append(wb_sb)
                        xt_sb = xp.tile([P, kb, CA], BF16, tag=f"xta_{b}", name=f"xta_{b}")
                        nc.scalar.dma_start(xt_sb[:], xta_d[:, b * kb:(b + 1) * kb, :])
                        xta_tiles.append(xt_sb)
                    w13_tiles[(slot, kind, j)] = None  # handled via w13_first
                    return
                sb = w13p.tile([P, KO, P], BF16, tag="w13", name=f"w13_{slot}_{kind}_{j}")
                nc.sync.dma_start(sb[:], src)
                w13_tiles[(slot, kind, j)] = sb

            def w13_slice(slot, kind, j, ko):
                sb = w13_tiles[(slot, kind, j)]
                if sb is None:
                    kb = KO // XB
                    return w13_first[ko // kb][:, ko % kb, :]
                return sb[:, ko, :]

            def dma_xtb(b):
                # B-slot tokens in 2 half-KO blocks, needed ~4 us after start
                kb = KO // 2
                xt_sb = xp.tile([P, kb, CB], BF16, tag=f"xtb_{b}", name=f"xtb_{b}")
                nc.scalar.dma_start(xt_sb[:], xtb_d[:, b * kb:(b + 1) * kb, :])
                xtb_tiles.append(xt_sb)

            w2a_tiles = {}

            def dma_w2a(j):
                sb = w2ap.tile([P, 2, NC1 * P], BF16, tag="w2a", name=f"w2a_{j}")
                nc.sync.dma_start(sb[:], w2a_d[j])
                w2a_tiles[j] = sb

            w2b_tiles = {}

            def dma_w2b(slot, jp):
                sb = w2bp.tile([P, 2, WB], BF16, tag=f"w2b_{slot}_{jp}", name=f"w2b_{slot}_{jp}")
                nc.sync.dma_start(sb[:], w2b_d[slot, jp])
                w2b_tiles[(slot, jp)] = sb

            def w2b_slice(slot, j, h):
                return w2b_tiles[(slot, j // 2)][:, j % 2, (h - NC1) * P:(h - NC1 + 1) * P]

            def xt_slice(ko, slot):
                if slot == 0:
                    kb = KO // XB
                    return xta_tiles[ko // kb][:, ko % kb, :]
                kb = KO // 2
                return xtb_tiles[ko // kb][:, ko % kb, :]

            # ---- priming: consumption-ordered, minimal instruction count
            xta_tiles, xtb_tiles = [], []
            dma_w13(0, 0, 0, blocked=True)  # + xtA blocks interleaved
            dma_w13(0, 1, 0)
            dma_xtb(0)
            dma_w13(1, 0, 0)
            dma_xtb(1)
            dma_w13(1, 1, 0)
            dma_w2a(0)
            dma_w2a(1)
            for slot in (0, 1):
                dma_w13(slot, 0, 1)
                dma_w13(slot, 1, 1)

            # persistent PSUM accumulators for the first NC1 output chunks
            acc = {}
            for slot, _, cw in SLOTS:
                for c in range(NC1):
                    acc[(slot, c)] = psacc.tile(
                        [P, cw], F32, tag=f"acc{slot}{c}", name=f"acc_{slot}_{c}"
                    )

            act_tiles = {}
            # ---- GU phase: gate/up + SwiGLU, slots interleaved per j
            for j in range(NJ):
                # prefetch group j+2, then pace w2b_A into the stream over
                # the back half of the GU loop
                if j + 2 < NJ:
                    for slot in (0, 1):
                        dma_w13(slot, 0, j + 2)
                        dma_w13(slot, 1, j + 2)
                    dma_w2a(j + 2)
                if j >= 8:
                    dma_w2b(0, j - 8)

                for slot, c0, cw in SLOTS:
                    g_ps = psgu.tile([P, cw], F32, tag="g", name=f"g_{slot}_{j}")
                    u_ps = psgu.tile([P, cw], F32, tag="u", name=f"u_{slot}_{j}")
                    for kind, ps in ((0, g_ps), (1, u_ps)):
                        for ko in range(KO):
                            nc.tensor.matmul(
                                ps[:],
                                w13_slice(slot, kind, j, ko),
                                xt_slice(ko, slot),
                                start=(ko == 0),
                                stop=(ko == KO - 1),
                            )
                    # silu(g)*u as sigmoid + 2 muls
                    s_sb = silup.tile([P, cw], F32, tag="s", name=f"s_{slot}_{j}")
                    nc.scalar.activation(s_sb[:], g_ps[:], act_fn.Sigmoid)
                    su_sb = silup.tile([P, cw], F32, tag="su", name=f"su_{slot}_{j}")
                    nc.vector.tensor_mul(su_sb[:], s_sb[:], u_ps[:])
                    a_sb = actp.tile([P, cw], BF16, tag=f"act_{slot}_{j}", name=f"act_{slot}_{j}")
                    nc.vector.tensor_mul(a_sb[:], su_sb[:], g_ps[:])
                    act_tiles[(slot, j)] = a_sb

                # fold down-proj chunks 0..NC1-1 for act[j-1] into this step
                if j >= 1:
                    for slot, _, cw in SLOTS:
                        for c in range(NC1):
                            nc.tensor.matmul(
                                acc[(slot, c)][:],
                                w2a_tiles[j - 1][:, slot, c * P:(c + 1) * P],
                                act_tiles[(slot, j - 1)][:],
                                start=(j - 1 == 0),
                                stop=False,
                            )

            def writeback(ps, h, slot, c0, cw, eng=None):
                o_sb = outp.tile([P, cw], BF16, tag="o", name=f"o_{slot}_{h}")
                nc.vector.tensor_copy(o_sb[:], ps[:])
                # gpsimd queue: keeps output writebacks off the (FIFO) input
                # weight-stream queue so they can't back up the PSUM drain.
                # Late chunks go on sync (stream empty by then; faster drain).
                (eng or nc.gpsimd).dma_start(y_d[h][:, c0:c0 + cw], o_sb[:])

            # finish the interleaved accumulators (act[NJ-1]) and drain them
            for slot, c0, cw in SLOTS:
                for c in range(NC1):
                    nc.tensor.matmul(
                        acc[(slot, c)][:],
                        w2a_tiles[NJ - 1][:, slot, c * P:(c + 1) * P],
                        act_tiles[(slot, NJ - 1)][:],
                        start=False,
                        stop=True,
                    )
            # w2b_B stream lands during the A down phase
            for jp in range(NJ // 2):
                dma_w2b(1, jp)
            for slot, c0, cw in SLOTS:
                for c in range(NC1):
                    writeback(acc[(slot, c)], c, slot, c0, cw)

            # ---- DOWN phase: remaining chunks, 6-wide PSUM passes per slot,
            # j ascending inside each chunk to match w2b arrival order
            for slot, c0, cw in SLOTS:
                tag_cycle = ["g", "g", "u", "u", f"acc{slot}0", f"acc{slot}1"]
                for hi, h in enumerate(range(NC1, NH)):
                    ps = (psgu if hi % 6 < 4 else psacc).tile(
                        [P, cw], F32, tag=tag_cycle[hi % 6], name=f"yd_{slot}_{h}"
                    )
                    for j in range(NJ):
                        nc.tensor.matmul(
                            ps[:],
                            w2b_slice(slot, j, h),
                            act_tiles[(slot, j)][:],
                            start=(j == 0),
                            stop=(j == NJ - 1),
                        )
                    writeback(ps, h, slot, c0, cw,
                              eng=nc.sync if slot == 1 else None)
    nc.compile()
    return nc


def _route(router_logits: np.ndarray, top_k: int):
    """Match jax.nn.softmax + jax.lax.top_k + renormalize (ties -> lower idx)."""
    p = router_logits.astype(np.float64)
    p = np.exp(p - p.max(axis=-1, keepdims=True))
    p /= p.sum(axis=-1, keepdims=True)
    order = np.argsort(-p, axis=-1, kind="stable")
    idx = order[:, :top_k]
    w = np.take_along_axis(p, idx, axis=-1)
    w /= w.sum(axis=-1, keepdims=True)
    return idx, w


def _pad4(n: int) -> int:
    return max(16, -(-n // 4) * 4)


def kernel(hidden_states, router_logits, W13, W2, top_k):
    global last_results
    top_k = int(top_k)
    hs = np.asarray(hidden_states, dtype=np.float32)
    T = hs.shape[0]
    idx, w = _route(np.asarray(router_logits, dtype=np.float32), top_k)

    tok_ids, tok_w = [], []
    for e in range(E):
        sel = idx == e  # [T, k]; at most one True per row
        rows = np.nonzero(sel.any(axis=-1))[0]
        tok_ids.append(rows)
        tok_w.append(w[sel].astype(np.float32))  # row-major -> token order

    # sort experts by load desc; pair big (slot A) with small (slot B)
    loads = np.array([len(r) for r in tok_ids])
    order = np.argsort(-loads, kind="stable")
    pairs = [(int(order[p]), int(order[7 - p])) for p in range(4)]
    CA = _pad4(max(loads[a] for a, _ in pairs))
    CB = _pad4(max(loads[b] for _, b in pairs))
    assert CA <= 512 and CB <= 512, "token capacity exceeds one PSUM bank"
    CT = CA + CB

    W13 = np.asarray(W13, dtype=np.float32)
    W2 = np.asarray(W2, dtype=np.float32)
    hsb = hs.astype(ml_dtypes.bfloat16)

    def w13_shard(e, hf):
        # [gate-half; up-half] rows -> [2NJ, P, KO, P] tiled, partition=h-col
        wg = W13[e][hf * IH:(hf + 1) * IH]
        wu = W13[e][I + hf * IH:I + (hf + 1) * IH]
        both = np.concatenate([wg, wu], axis=0).astype(ml_dtypes.bfloat16)
        return np.ascontiguousarray(
            both.reshape(2 * NJ, P, KO, P).transpose(0, 3, 2, 1)
        )

    def w2_shard(e, hf):
        # contraction rows i within the half -> [NJ, P, H]
        w2h = W2[e][:, hf * IH:(hf + 1) * IH].astype(ml_dtypes.bfloat16)
        return np.ascontiguousarray(w2h.reshape(H, NJ, P).transpose(1, 2, 0))

    in_maps = []
    for core in range(N_CORES):
        p, hf = core // 2, core % 2
        ea, eb = pairs[p]
        def xt_arr(e, cap):
            xt = np.zeros((P, KO, cap), dtype=ml_dtypes.bfloat16)
            rows = tok_ids[e]
            n_e = len(rows)
            if n_e:
                xg = hsb[rows]  # [n_e, H]
                xt[:, :, :n_e] = xg.T.reshape(KO, P, n_e).transpose(1, 0, 2)
            return xt

        w2 = np.stack([w2_shard(ea, hf), w2_shard(eb, hf)])  # [2, NJ, P, H]
        in_maps.append({
            "xta": xt_arr(ea, CA),
            "xtb": xt_arr(eb, CB),
            "w13": np.stack([w13_shard(ea, hf), w13_shard(eb, hf)]),
            # [NJ, P, 2, NC1*P]: both slots packed per j
            "w2a": np.ascontiguousarray(w2[:, :, :, :NC1 * P].transpose(1, 2, 0, 3)),
            # [2, NJ//2, P, 2, WB]: j-pairs packed per slot
            "w2b": np.ascontiguousarray(
                w2[:, :, :, NC1 * P:].reshape(2, NJ // 2, 2, P, WB).transpose(0, 1, 3, 2, 4)
            ),
        })

    nc = _build_nc(CA, CB)
    res = run_bass_kernel_spmd(
        nc,
        in_maps,
        list(range(N_CORES)),
        trace=bool(os.environ.get("MOE_TRACE")),
        tmpdir=os.environ.get("MOE_TRACE_DIR") or None,
    )
    last_results = res

    out = np.zeros((T, H), dtype=np.float32)
    for p in range(4):
        ea, eb = pairs[p]
        y0 = res.results[2 * p]["y"].reshape(H, CT).astype(np.float32)
        y1 = res.results[2 * p + 1]["y"].reshape(H, CT).astype(np.float32)
        ysum = y0 + y1
        for (e, c0) in ((ea, 0), (eb, CA)):
            rows = tok_ids[e]
            n_e = len(rows)
            if n_e:
                out[rows] += ysum[:, c0:c0 + n_e].T * tok_w[e][:, None]
    return out
